# revision 2
# baseline (speedup 1.0000x reference)
"""Batched matrix-attention scores kernel for Trainium2 (8 NeuronCores).

Computes scores[b, i, j] = sum_d m1[b, i, d] * m2[b, j, d]
  (i.e. jnp.einsum('bid,bjd->bij', matrix_1, matrix_2))
with B=16, R1=R2=2048, D=256, fp32.

Sharding: data-parallel over batch — 2 batches per core on 8 cores.

Per-core kernel structure (per batch):
  1. DMA both operand matrices [2048, 256] into SBUF in natural layout.
  2. PE-transpose (matmul with identity, fp32) into [D-on-partitions]
     layout: mT[d, dc, row] tiles, since the tensor engine contracts
     over the partition dimension.
  3. For each 128-row i-tile: 8 fp32 matmuls (4 j-chunks x 2 d-chunks,
     N=512 into one PSUM bank each), evacuate PSUM->SBUF, DMA the
     [128, 2048] row-block to HBM.
"""

from contextlib import ExitStack

import numpy as np

import concourse.bass as bass
import concourse.mybir as mybir
import concourse.tile as tile
from concourse import bacc
from concourse.bass_utils import run_bass_kernel_spmd
from concourse.masks import make_identity

F32 = mybir.dt.float32

NCORES = 8
B, R1, R2, D = 16, 2048, 2048, 256
BPC = B // NCORES  # batches per core
P = 128
NJ_TILE = 512  # matmul free dim (one fp32 PSUM bank)
NJ = R2 // NJ_TILE  # j-chunks per row-block
NT = R1 // P  # 128-row tiles per batch
DC = D // P  # contraction chunks


def _build_tile_kernel(ctx: ExitStack, tc: tile.TileContext, m1, m2, out):
    nc = tc.nc

    const_pool = ctx.enter_context(tc.tile_pool(name="const", bufs=1))
    ident = const_pool.tile([P, P], F32)
    make_identity(nc, ident)

    nat_pool = ctx.enter_context(tc.tile_pool(name="nat", bufs=2))
    mt_pool = ctx.enter_context(tc.tile_pool(name="mt", bufs=2))
    tpsum = ctx.enter_context(tc.tile_pool(name="tpsum", bufs=2, space="PSUM"))
    mpsum = ctx.enter_context(tc.tile_pool(name="mpsum", bufs=4, space="PSUM"))
    outp = ctx.enter_context(tc.tile_pool(name="outp", bufs=3))

    for b in range(BPC):
        # ---- load + transpose both operands into [d, dc, row] layout ----
        mts = []
        for name, src in (("m1T", m1), ("m2T", m2)):
            nat = nat_pool.tile([P, NT, D], F32, tag="nat")
            nc.sync.dma_start(nat, src[b].rearrange("(o p) d -> p o d", p=P))
            mt = mt_pool.tile([P, DC, R1], F32, tag=name)
            for o in range(NT):
                for dc in range(DC):
                    ps = tpsum.tile([P, P], F32, tag="tps")
                    nc.tensor.transpose(
                        ps, nat[:, o, dc * P : (dc + 1) * P], ident
                    )
                    nc.any.tensor_copy(mt[:, dc, o * P : (o + 1) * P], ps)
            mts.append(mt)
        m1T, m2T = mts

        # ---- matmuls: one 128-row block of the output at a time ----
        for it in range(NT):
            stage = outp.tile([P, R2], F32, tag="stage")
            for jc in range(NJ):
                ps = mpsum.tile([P, NJ_TILE], F32, tag="mps")
                for dc in range(DC):
                    nc.tensor.matmul(
                        ps,
                        m1T[:, dc, it * P : (it + 1) * P],
                        m2T[:, dc, jc * NJ_TILE : (jc + 1) * NJ_TILE],
                        start=(dc == 0),
                        stop=(dc == DC - 1),
                    )
                nc.any.tensor_copy(stage[:, jc * NJ_TILE : (jc + 1) * NJ_TILE], ps)
            nc.sync.dma_start(out[b, it * P : (it + 1) * P, :], stage)


_NC_CACHE = None


def _build():
    global _NC_CACHE
    if _NC_CACHE is not None:
        return _NC_CACHE
    nc = bacc.Bacc(
        "TRN2", target_bir_lowering=False, debug=False, num_devices=NCORES
    )
    m1 = nc.dram_tensor("m1", [BPC, R1, D], F32, kind="ExternalInput").ap()
    m2 = nc.dram_tensor("m2", [BPC, R2, D], F32, kind="ExternalInput").ap()
    out = nc.dram_tensor("out", [BPC, R1, R2], F32, kind="ExternalOutput").ap()
    with tile.TileContext(nc) as tc:
        with ExitStack() as ctx:
            _build_tile_kernel(ctx, tc, m1, m2, out)
    nc.compile()
    _NC_CACHE = nc
    return nc


def kernel(matrix_1: np.ndarray, matrix_2: np.ndarray, **run_kwargs) -> np.ndarray:
    m1 = np.ascontiguousarray(np.asarray(matrix_1, dtype=np.float32))
    m2 = np.ascontiguousarray(np.asarray(matrix_2, dtype=np.float32))
    assert m1.shape == (B, R1, D) and m2.shape == (B, R2, D)

    nc = _build()
    in_maps = [
        {
            "m1": m1[i * BPC : (i + 1) * BPC],
            "m2": m2[i * BPC : (i + 1) * BPC],
        }
        for i in range(NCORES)
    ]
    res = run_bass_kernel_spmd(
        nc, in_maps, core_ids=list(range(NCORES)), **run_kwargs
    )
    out = np.empty((B, R1, R2), dtype=np.float32)
    for i in range(NCORES):
        out[i * BPC : (i + 1) * BPC] = res.results[i]["out"]
    if run_kwargs:
        kernel.last_result = res
    return out


# revision 4
# speedup vs baseline: 2.1185x; 2.1185x over previous
"""Batched matrix-attention scores kernel for Trainium2 (8 NeuronCores).

Computes scores[b, i, j] = sum_d m1[b, i, d] * m2[b, j, d]
  (i.e. jnp.einsum('bid,bjd->bij', matrix_1, matrix_2))
with B=16, R1=R2=2048, D=256, fp32 in/out.

Sharding: data-parallel over batch — 2 batches per core on 8 cores.

Per-core kernel structure (per batch):
  1. DMA both operand matrices [2048, 256] into SBUF in natural layout.
  2. PE-transpose (matmul with identity) into D-on-partitions layout:
     mT[d, dc, row], since the tensor engine contracts over partitions.
  3. For each 128-row i-tile: 8 matmuls (2 d-chunks x 4 j-chunks of
     N=512, one fp32 PSUM bank each), evacuate PSUM->SBUF on DVE, DMA
     the [128, 2048] row-block to HBM.

Operands use dt.float32r (fp32 bits, full-rate single-pass PE matmul;
~2^-11 input mantissa truncation) — ~4x the fp32 matmul rate.
Accumulation stays fp32 in PSUM; output is exact fp32 layout.
"""

from contextlib import ExitStack

import numpy as np

import concourse.bass as bass
import concourse.mybir as mybir
import concourse.tile as tile
from concourse import bacc
from concourse.bass_utils import run_bass_kernel_spmd

F32 = mybir.dt.float32
F32R = mybir.dt.float32r

NCORES = 8
B, R1, R2, D = 16, 2048, 2048, 256
BPC = B // NCORES  # batches per core
P = 128
NJ_TILE = 512  # matmul free dim (one fp32 PSUM bank)
NJ = R2 // NJ_TILE  # j-chunks per row-block
NT = R1 // P  # 128-row tiles per batch
DC = D // P  # contraction chunks


def _build_tile_kernel(ctx: ExitStack, tc: tile.TileContext, m1, m2, ident_in, out):
    nc = tc.nc

    const_pool = ctx.enter_context(tc.tile_pool(name="const", bufs=1))
    ident = const_pool.tile([P, P], F32R)
    nc.sync.dma_start(ident, ident_in)

    nat_pool = ctx.enter_context(tc.tile_pool(name="nat", bufs=2))
    mt_pool = ctx.enter_context(tc.tile_pool(name="mt", bufs=2))
    tpsum = ctx.enter_context(tc.tile_pool(name="tpsum", bufs=2, space="PSUM"))
    mpsum = ctx.enter_context(tc.tile_pool(name="mpsum", bufs=6, space="PSUM"))
    outp = ctx.enter_context(tc.tile_pool(name="outp", bufs=3))

    for b in range(BPC):
        # ---- load + transpose both operands into [d, dc, row] layout ----
        mts = []
        for name, src in (("m2T", m2), ("m1T", m1)):
            nat = nat_pool.tile([P, NT, D], F32R, tag="nat")
            nc.sync.dma_start(nat, src[b].rearrange("(o p) d -> p o d", p=P))
            mt = mt_pool.tile([P, DC, R1], F32R, tag=name)
            for o in range(NT):
                for dc in range(DC):
                    ps = tpsum.tile([P, P], F32R, tag="tps")
                    nc.tensor.transpose(
                        ps, nat[:, o, dc * P : (dc + 1) * P], ident
                    )
                    nc.vector.tensor_copy(mt[:, dc, o * P : (o + 1) * P], ps)
            mts.append(mt)
        m2T, m1T = mts

        # ---- matmuls: one 128-row block of the output at a time ----
        for it in range(NT):
            stage = outp.tile([P, R2], F32, tag="stage")
            pss = [
                mpsum.tile([P, NJ_TILE], F32, tag="mps", name=f"mps_{b}_{it}_{jc}")
                for jc in range(NJ)
            ]
            for dc in range(DC):
                for jc in range(NJ):
                    nc.tensor.matmul(
                        pss[jc],
                        m1T[:, dc, it * P : (it + 1) * P],
                        m2T[:, dc, jc * NJ_TILE : (jc + 1) * NJ_TILE],
                        start=(dc == 0),
                        stop=(dc == DC - 1),
                    )
            for jc in range(NJ):
                nc.vector.tensor_copy(
                    stage[:, jc * NJ_TILE : (jc + 1) * NJ_TILE], pss[jc]
                )
            nc.sync.dma_start(out[b, it * P : (it + 1) * P, :], stage)


_NC_CACHE = None


def _build():
    global _NC_CACHE
    if _NC_CACHE is not None:
        return _NC_CACHE
    nc = bacc.Bacc(
        "TRN2", target_bir_lowering=False, debug=False, num_devices=NCORES
    )
    m1 = nc.dram_tensor("m1", [BPC, R1, D], F32R, kind="ExternalInput").ap()
    m2 = nc.dram_tensor("m2", [BPC, R2, D], F32R, kind="ExternalInput").ap()
    ident_in = nc.dram_tensor("ident", [P, P], F32R, kind="ExternalInput").ap()
    out = nc.dram_tensor("out", [BPC, R1, R2], F32, kind="ExternalOutput").ap()
    with tile.TileContext(nc) as tc:
        with ExitStack() as ctx:
            _build_tile_kernel(ctx, tc, m1, m2, ident_in, out)
    nc.compile()
    _NC_CACHE = nc
    return nc


def kernel(matrix_1: np.ndarray, matrix_2: np.ndarray, **run_kwargs) -> np.ndarray:
    m1 = np.ascontiguousarray(np.asarray(matrix_1, dtype=np.float32))
    m2 = np.ascontiguousarray(np.asarray(matrix_2, dtype=np.float32))
    assert m1.shape == (B, R1, D) and m2.shape == (B, R2, D)

    nc = _build()
    eye = np.eye(P, dtype=np.float32)
    in_maps = [
        {
            "m1": m1[i * BPC : (i + 1) * BPC],
            "m2": m2[i * BPC : (i + 1) * BPC],
            "ident": eye,
        }
        for i in range(NCORES)
    ]
    res = run_bass_kernel_spmd(
        nc, in_maps, core_ids=list(range(NCORES)), **run_kwargs
    )
    out = np.empty((B, R1, R2), dtype=np.float32)
    for i in range(NCORES):
        out[i * BPC : (i + 1) * BPC] = res.results[i]["out"]
    if run_kwargs:
        kernel.last_result = res
    return out
